# revision 4
# baseline (speedup 1.0000x reference)
"""AttentionHead kernel for 8 Trainium2 NeuronCores.

Problem: B=4, S=2048, DIN=1024, DOUT=128 single-head attention with a
key-padding mask and a sqrt(S) score scale (see the module reference).

Sharding: 8 cores = 4 batches x 2 query-halves. The key-padding mask is
known host-side and masks ~half the keys, so each core receives only the
COMPACTED unmasked keys of its batch (padded to KC*128 with zeros and a
-60 exp bias, numerically identical to the reference softmax since
masked keys contribute exp(-1e9)=0 there and exp(-60)~9e-27 here).
This nearly halves the K/V projections, the score matmuls, the exp
stream, and the context matmuls versus the full-S kernel.

The query-side x is shipped in fp8e4m3 (wq pre-scaled by 64 so its
+-1/32 entries land in fp8 normal range; the 1/64 is folded into the
exp scale). Q-projection matmuls run fp8; everything else runs fp16
operands with fp32 PSUM accumulation. Measured end-to-end rel err
~2e-3 (fp8 Q path) vs the 2e-2 gate.

Per-core dataflow:
  1. Warmup matmuls (HAM un-throttle) while the first DMAs land.
  2. xq8 [1024d, 1024q] (one DMA) -> Q^T proj (fp8); xk [1024d, 1152k]
     fp16 (one DMA) -> K^T proj; PSUM->SBUF copies w/ bias on DVE+ACT.
  3. Per k-tile: scores^T[k,q] = K^T_tile.T @ Q^T (two 512-col matmuls
     into a 2-bank PSUM tile), one [128,1024] exp on ACT (mask/pad bias
     per-partition, scale/64 folded in), V-proj tile interleaved so the
     PE stays ahead of ACT's exp stream.
  4. V in natural [s,o] layout via xk-stationary matmuls with a ones
     column appended so the softmax denominator rides the context
     matmul; V bias added via a partition-broadcast tile.
  5. context[q, o|den] accumulates KC k-tile matmuls per 128-query
     tile; normalize = DVE reciprocal + per-partition multiply; output
     DMA'd in 3 chunks on otherwise-idle queues as tiles complete.
"""

import sys

for _p in ("/opt/trn_rl_repo", "/root/.axon_site",
           "/root/.axon_site/_ro/trn_rl_repo", "/root/.axon_site/_ro/pypackages"):
    if _p not in sys.path:
        sys.path.insert(0, _p)

import numpy as np

B, S, DIN, DOUT = 4, 2048, 1024, 128
NCORES = 8
SH = S // 2          # seq half (query rows per core)
DC = DIN // 128      # d chunks (8)
KC = 9               # compacted key tiles of 128 (capacity 1152 >= max
                     # unmasked keys; grown dynamically if ever exceeded)
WQ_SCALE = 64.0      # host-side wq multiplier (fp8 subnormal avoidance)
SCALE = 1.0 / float(np.sqrt(np.float32(S)))
MASK_BIAS = -60.0    # exp(-60) ~ 8.8e-27: numerically zero vs unmasked sum

_PROGRAMS = {}


def _build_program(reps=1, kc=KC):
    import concourse.bass as bass
    import concourse.mybir as mybir
    import concourse.tile as tile
    from concourse import bacc
    from contextlib import ExitStack

    f32 = mybir.dt.float32
    f16 = mybir.dt.float16
    f8 = mybir.dt.float8e4
    KCAP = kc * 128
    CBW = 3 + kc  # bq | bk | bv | mbias columns

    nc = bacc.Bacc(None, target_bir_lowering=False)

    xq8_d = nc.dram_tensor("xq8", [DIN, SH], f8, kind="ExternalInput")
    xk_d = nc.dram_tensor("xk", [DIN, KCAP], f16, kind="ExternalInput")
    wq8_d = nc.dram_tensor("wq8", [DIN, DOUT], f8, kind="ExternalInput")
    w16_d = nc.dram_tensor("w16", [DIN, 2 * DOUT], f16, kind="ExternalInput")
    cb_d = nc.dram_tensor("cb", [128, CBW], f32, kind="ExternalInput")
    ones_d = nc.dram_tensor("ones", [128, 4], f16, kind="ExternalInput")
    bvv_d = nc.dram_tensor("bvv", [DOUT, 1], f32, kind="ExternalInput")
    out_d = nc.dram_tensor("out", [SH, DOUT], f32, kind="ExternalOutput")

    with ExitStack() as ctx:
        tc = ctx.enter_context(tile.TileContext(nc))
        consts = ctx.enter_context(tc.tile_pool(name="consts", bufs=1))
        xp = ctx.enter_context(tc.tile_pool(name="xp", bufs=1))
        kqv = ctx.enter_context(tc.tile_pool(name="kqv", bufs=1))
        vnp = ctx.enter_context(tc.tile_pool(name="vnp", bufs=kc))
        ptp = ctx.enter_context(tc.tile_pool(name="ptp", bufs=kc))
        outp = ctx.enter_context(tc.tile_pool(name="outp", bufs=1))
        misc = ctx.enter_context(tc.tile_pool(name="misc", bufs=8))

        psA = ctx.enter_context(tc.tile_pool(name="psA", bufs=3, space="PSUM"))
        psM = ctx.enter_context(tc.tile_pool(name="psM", bufs=2, space="PSUM"))

        def body():
            # HAM warmup: dummy matmuls with no DMA dependency keep the PE
            # busy (and un-throttled) while the first DMAs land.
            dummy = misc.tile([128, 256], f16, tag="dummy", name="dummy")
            nc.vector.memset(dummy, 0.5)
            for i in range(12):
                psw = psM.tile([128, 132], f32, tag="psM", name=f"warm{i}")
                nc.tensor.matmul(psw[:, 0:128], dummy[:, 0:128],
                                 dummy[:, 0:128], start=True, stop=True)

            # ---- DMAs: coarse-grained (HWDGE config is ~630ns each,
            # serialized), wq8/xq8 first so the Q path starts early.
            wq8_sb = consts.tile([128, DC, DOUT], f8, tag="wq8", name="wq8")
            nc.scalar.dma_start(wq8_sb, wq8_d.rearrange("(c p) o -> p c o", p=128))
            xq8_sb = xp.tile([128, DC, SH], f8, tag="xq8", name="xq8")
            nc.sync.dma_start(xq8_sb, xq8_d.rearrange("(c p) s -> p c s", p=128))
            w16_sb = consts.tile([128, DC, 2 * DOUT], f16, tag="w16", name="w16")
            nc.scalar.dma_start(w16_sb, w16_d.rearrange("(c p) o -> p c o", p=128))
            xk_sb = xp.tile([128, DC, KCAP], f16, tag="xk", name="xk")
            nc.sync.dma_start(xk_sb, xk_d.rearrange("(c p) s -> p c s", p=128))
            cb_sb = consts.tile([128, CBW], f32, tag="cb", name="cb")
            nc.scalar.dma_start(cb_sb, cb_d[:, :])
            ones_sb = consts.tile([128, 4], f16, tag="ones", name="ones")
            nc.scalar.dma_start(ones_sb, ones_d[:, :])
            bv_bc = consts.tile([128, DOUT], f32, tag="bv_bc", name="bv_bc")
            nc.gpsimd.dma_start(
                out=bv_bc,
                in_=bass.AP(tensor=bvv_d, offset=0, ap=[[0, 128], [1, DOUT]]))
            bq = cb_sb[:, 0:1]
            bk = cb_sb[:, 1:2]

            KT = kqv.tile([128, KCAP], f16, tag="KT", name="KT")
            QT = kqv.tile([128, SH], f16, tag="QT", name="QT")

            # Q^T projection (fp8 operands; PSUM f32).
            psQ = psA.tile([128, 1024], f32, tag="psA", name="psQ")
            for c in range(DC):
                for sb in range(2):
                    nc.tensor.matmul(
                        psQ[:, sb * 512:(sb + 1) * 512],
                        wq8_sb[:, c, :],
                        xq8_sb[:, c, sb * 512:(sb + 1) * 512],
                        start=(c == 0), stop=(c == DC - 1))
            # K^T projection (fp16): 1024 cols in psA, 128-col tail in psM.
            psKa = psA.tile([128, 1024], f32, tag="psA", name="psKa")
            psKb = psM.tile([128, 132], f32, tag="psM", name="psKb")
            for c in range(DC):
                for sb in range(2):
                    nc.tensor.matmul(
                        psKa[:, sb * 512:(sb + 1) * 512],
                        w16_sb[:, c, 0:DOUT],
                        xk_sb[:, c, sb * 512:(sb + 1) * 512],
                        start=(c == 0), stop=(c == DC - 1))
                nc.tensor.matmul(
                    psKb[:, 0:128],
                    w16_sb[:, c, 0:DOUT],
                    xk_sb[:, c, 1024:KCAP],
                    start=(c == 0), stop=(c == DC - 1))

            # PSUM -> SBUF copies (+bias, fp16 cast) split across DVE/ACT.
            nc.vector.tensor_scalar_add(QT[:, 0:512], psQ[:, 0:512], bq)
            nc.scalar.activation(
                QT[:, 512:1024], psQ[:, 512:1024],
                mybir.ActivationFunctionType.Identity, bias=bq)
            nc.vector.tensor_scalar_add(KT[:, 0:512], psKa[:, 0:512], bk)
            nc.scalar.activation(
                KT[:, 512:1024], psKa[:, 512:1024],
                mybir.ActivationFunctionType.Identity, bias=bk)
            nc.vector.tensor_scalar_add(KT[:, 1024:KCAP], psKb[:, 0:128], bk)

            # scores^T + exp per k-tile, V-proj tiles interleaved so the
            # PE keeps feeding ACT's exp stream without long stalls.
            PT = [None] * kc
            VN = [None] * kc

            def emit_scores(kt):
                pss = psA.tile([128, 1024], f32, tag="psA", name=f"psS{kt}")
                for qh in range(2):
                    nc.tensor.matmul(
                        pss[:, qh * 512:(qh + 1) * 512],
                        KT[:, kt * 128:(kt + 1) * 128],
                        QT[:, qh * 512:(qh + 1) * 512],
                        start=True, stop=True)
                pt = ptp.tile([128, 1024], f16, tag="pt", name=f"pt{kt}")
                nc.scalar.activation(
                    pt, pss, mybir.ActivationFunctionType.Exp,
                    bias=cb_sb[:, 3 + kt:4 + kt], scale=SCALE / WQ_SCALE)
                PT[kt] = pt

            def emit_v(kt):
                psv = psM.tile([128, 132], f32, tag="psM", name=f"psV{kt}")
                for c in range(DC):
                    nc.tensor.matmul(
                        psv[:, 0:128],
                        xk_sb[:, c, kt * 128:(kt + 1) * 128],
                        w16_sb[:, c, DOUT:2 * DOUT],
                        start=(c == 0), stop=(c == DC - 1))
                vt = vnp.tile([128, 132], f16, tag="vn", name=f"vn{kt}")
                nc.vector.tensor_tensor(
                    vt[:, 0:128], psv[:, 0:128], bv_bc, mybir.AluOpType.add)
                nc.vector.tensor_copy(out=vt[:, 128:132], in_=ones_sb)
                VN[kt] = vt

            emit_scores(0)
            emit_scores(1)
            for kt in range(2, kc):
                emit_v(kt - 2)
                emit_scores(kt)
            emit_v(kc - 2)
            emit_v(kc - 1)

            # ---- context + normalize -----------------------------------
            out_r = out_d.rearrange("(t p) o -> p t o", p=128)
            OUT = outp.tile([128, SH // 128, DOUT], f32, tag="out")
            n_q2 = SH // 128
            for q2 in range(n_q2):
                if q2 in (0, 1, 5, 6):
                    psc = psM.tile([128, 132], f32, tag="psM", name=f"psC{q2}")
                else:
                    psc = psA.tile([128, 1024], f32, tag="psA",
                                   name=f"psC{q2}")[:, 0:132]
                for kt in range(kc):
                    nc.tensor.matmul(
                        psc,
                        PT[kt][:, q2 * 128:(q2 + 1) * 128],
                        VN[kt][:, 0:132],
                        start=(kt == 0), stop=(kt == kc - 1))
                drec = misc.tile([128, 1], f32, tag="drec", name=f"drec{q2}")
                nc.vector.reciprocal(drec, psc[:, 128:129])
                nc.vector.tensor_scalar_mul(
                    OUT[:, q2, :], psc[:, 0:128], drec)
                if q2 == 3:
                    nc.gpsimd.dma_start(out_r[:, 0:4, :], OUT[:, 0:4, :])
                elif q2 == 6:
                    nc.sync.dma_start(out_r[:, 4:7, :], OUT[:, 4:7, :])
                elif q2 == 7:
                    nc.sync.dma_start(out_r[:, 7:8, :], OUT[:, 7:8, :])

        if reps == 1:
            body()
        else:
            with tc.For_i(0, reps, 1):
                body()

    nc.finalize()
    return nc


def _get_program(kc=KC):
    if kc not in _PROGRAMS:
        _PROGRAMS[kc] = _build_program(reps=1, kc=kc)
    return _PROGRAMS[kc]


def _stage_inputs(inputs, kc=None):
    import ml_dtypes

    f8 = ml_dtypes.float8_e4m3

    x = np.asarray(inputs["input_tensor"], dtype=np.float32)
    mask = np.asarray(inputs["attention_mask"]).astype(bool)
    keys = [np.nonzero(~mask[b, 0])[0] for b in range(B)]
    if kc is None:
        need = max(1, -(-max(len(k) for k in keys) // 128))
        kc = max(KC, need)
    KCAP = kc * 128
    CBW = 3 + kc

    ws = {k: np.asarray(inputs[k], dtype=np.float32)
          for k in ("wq", "wk", "wv")}
    bs = {k: np.asarray(inputs[k], dtype=np.float32) for k in ("bq", "bk", "bv")}
    wq8 = np.ascontiguousarray(ws["wq"].T * WQ_SCALE).astype(f8)
    w16 = np.ascontiguousarray(
        np.concatenate([ws["wk"].T, ws["wv"].T], axis=1)).astype(np.float16)

    per_batch = {}
    for b in range(B):
        kb = keys[b]
        nk = len(kb)
        assert nk <= KCAP, (nk, KCAP)
        xk = np.zeros((DIN, KCAP), dtype=np.float16)
        xk[:, :nk] = x[b][kb, :].T
        cb = np.zeros((128, CBW), dtype=np.float32)
        cb[:, 0] = bs["bq"] * WQ_SCALE
        cb[:, 1] = bs["bk"]
        cb[:, 2] = bs["bv"]
        mb = np.full(KCAP, np.float32(MASK_BIAS))
        mb[:nk] = 0.0
        cb[:, 3:] = mb.reshape(kc, 128).T
        per_batch[b] = (xk, cb)

    ones = np.ones((128, 4), dtype=np.float16)
    in_maps = []
    for c in range(NCORES):
        b, h = divmod(c, 2)
        xk, cb = per_batch[b]
        xq8 = np.ascontiguousarray(
            x[b][h * SH:(h + 1) * SH, :].T).astype(f8)
        in_maps.append({
            "xq8": xq8, "xk": xk,
            "wq8": wq8, "w16": w16,
            "cb": cb, "ones": ones,
            "bvv": bs["bv"].reshape(DOUT, 1),
        })
    return in_maps


def run(inputs, **spmd_kwargs):
    """Run on 8 cores; returns (full_output, BassKernelResults)."""
    from concourse import bass_utils

    mask = np.asarray(inputs["attention_mask"]).astype(bool)
    max_nk = max(int((~mask[b, 0]).sum()) for b in range(B))
    kc = max(KC, -(-max_nk // 128))
    nc = _get_program(kc)
    in_maps = _stage_inputs(inputs, kc=kc)
    res = bass_utils.run_bass_kernel_spmd(
        nc, in_maps, core_ids=list(range(NCORES)), **spmd_kwargs)
    out = np.empty((B, S, DOUT), dtype=np.float32)
    for c in range(NCORES):
        b, h = divmod(c, 2)
        out[b, h * SH:(h + 1) * SH, :] = res.results[c]["out"]
    return out, res


def kernel(**inputs) -> np.ndarray:
    return run(inputs)[0]
